# revision 11
# baseline (speedup 1.0000x reference)
"""Trainium2 Bass kernel for a 3-layer LSTM with autoregressive rollout.

Model (hardcoded shapes):
  x [1024, 100, 256] -> 3 stacked LSTM layers (512 units) warmup over the
  100 input steps, then a 100-step autoregressive rollout through a dense
  head (512 -> 256). Output [1024, 100, 256] float32.

Strategy: data-parallel over batch across 8 NeuronCores (128 rows/core =
SBUF partition count). All weights resident in SBUF as bf16; activations
transposed chunks are the stationary matmul operand, weights stream
(N=512 per PSUM bank), fp32 PSUM accumulate; gates via ScalarE
sigmoid/tanh; c/h updates on VectorE in fp32; h re-transposed with PE
transposes. Fully unrolled, software-pipelined emission: each cell's
recurrent (U-side) matmuls are split around the previous cell's
transposes so the gate->h->transpose latency chain is always covered by
independent PE work.
"""

import numpy as np
import ml_dtypes

BF16 = ml_dtypes.bfloat16

B, T, S, D, U, G = 1024, 100, 100, 256, 512, 2048
NCORES, BC = 8, 128
KD, KU = D // 128, U // 128  # 2, 4

_CACHE = {}


def _build(Twarm, Sout, with_bias):
    from contextlib import ExitStack

    from concourse import bacc
    import concourse.mybir as mybir
    import concourse.tile as tile

    f32 = mybir.dt.float32
    bf16 = mybir.dt.bfloat16
    AF = mybir.ActivationFunctionType

    nc = bacc.Bacc("TRN2", target_bir_lowering=False, num_devices=NCORES)

    xT = nc.dram_tensor("xT", [Twarm, KD, 128, BC], bf16, kind="ExternalInput")
    wx_d = [
        nc.dram_tensor(f"w{l}", [KD if l == 0 else KU, 128, G], bf16,
                       kind="ExternalInput")
        for l in range(3)
    ]
    uw_d = [
        nc.dram_tensor(f"u{l}", [KU, 128, G], bf16, kind="ExternalInput")
        for l in range(3)
    ]
    wd_d = nc.dram_tensor("wd", [KU, 128, D], bf16, kind="ExternalInput")
    id_d = nc.dram_tensor("ident", [128, 128], bf16, kind="ExternalInput")
    if with_bias:
        bl_d = [
            nc.dram_tensor(f"b{l}", [1, G], bf16, kind="ExternalInput")
            for l in range(3)
        ]
        bd_d = nc.dram_tensor("bd", [1, D], bf16, kind="ExternalInput")
    out = nc.dram_tensor("out", [BC, Sout, D], f32, kind="ExternalOutput")

    with tile.TileContext(nc) as tc, ExitStack() as ctx:
        const = ctx.enter_context(tc.tile_pool(name="const", bufs=1))
        state = ctx.enter_context(tc.tile_pool(name="state", bufs=1))
        gp = ctx.enter_context(tc.tile_pool(name="gates", bufs=3))
        xp = ctx.enter_context(tc.tile_pool(name="xp", bufs=2))
        op = ctx.enter_context(tc.tile_pool(name="outp", bufs=2))
        zp = ctx.enter_context(tc.tile_pool(name="z", bufs=6, space="PSUM"))
        tp_ = ctx.enter_context(tc.tile_pool(name="tp", bufs=1, space="PSUM"))
        dp = ctx.enter_context(tc.tile_pool(name="dp", bufs=1, space="PSUM"))

        def load_w(dram, nk, width, tag):
            t = const.tile([128, nk * width], bf16, name=tag, tag=tag)
            for k in range(nk):
                nc.sync.dma_start(out=t[:, k * width:(k + 1) * width],
                                  in_=dram[k])
            return t

        wx_sb = [load_w(wx_d[l], KD if l == 0 else KU, G, f"wx{l}")
                 for l in range(3)]
        u_sb = [load_w(uw_d[l], KU, G, f"uw{l}") for l in range(3)]
        wd_sb = load_w(wd_d, KU, D, "wdw")
        ident = const.tile([128, 128], bf16)
        nc.sync.dma_start(out=ident[:], in_=id_d[:])
        if with_bias:
            ones = const.tile([1, 128], bf16)
            nc.vector.memset(ones[:], 1.0)
            b_sb = [const.tile([1, G], bf16, name=f"bsb{l}", tag=f"bsb{l}")
                    for l in range(3)]
            for l in range(3):
                nc.sync.dma_start(out=b_sb[l][:], in_=bl_d[l][:])
            bd_sb = const.tile([1, D], bf16)
            nc.sync.dma_start(out=bd_sb[:], in_=bd_d[:])

        hT = [state.tile([128, U], bf16, name=f"hT{l}", tag=f"hT{l}")
              for l in range(3)]
        cst = [state.tile([128, U], f32, name=f"cst{l}", tag=f"cst{l}")
              for l in range(3)]
        for l in range(3):
            nc.vector.memset(hT[l][:], 0.0)
            nc.vector.memset(cst[l][:], 0.0)
        pred0T = state.tile([128, D], bf16)

        def cell_begin(l):
            # 4 PSUM gate accumulators (i, f, g, o), single rotating tag
            return {"z": [zp.tile([128, 512], f32, name=f"zq{q}", tag="zq")
                          for q in range(4)],
                    "g": [None] * 4, "hb": None}

        def cell_U(cc, l, qs):
            for q in qs:
                for k in range(KU):
                    nc.tensor.matmul(
                        cc["z"][q][:], lhsT=hT[l][:, k * 128:(k + 1) * 128],
                        rhs=u_sb[l][:, k * G + q * 512: k * G + q * 512 + 512],
                        start=(k == 0), stop=False, skip_group_check=True)

        def cell_X(cc, l, x_tile, x_nk, x_w):
            for q in range(4):
                for k in range(x_nk):
                    is_last = (k == x_nk - 1) and not with_bias
                    nc.tensor.matmul(
                        cc["z"][q][:], lhsT=x_tile[:, k * 128:(k + 1) * 128],
                        rhs=x_w[:, k * G + q * 512: k * G + q * 512 + 512],
                        start=False, stop=is_last, skip_group_check=True)
                if with_bias:
                    nc.tensor.matmul(cc["z"][q][:], lhsT=ones[:],
                                     rhs=b_sb[l][:, q * 512:(q + 1) * 512],
                                     start=False, stop=True,
                                     skip_group_check=True)
                g_t = gp.tile([128, 512], f32, name=f"gate{q}", tag=f"gate{q}")
                nc.scalar.activation(out=g_t[:], in_=cc["z"][q][:],
                                     func=(AF.Tanh if q == 2 else AF.Sigmoid))
                cc["g"][q] = g_t

        def cell_tail(cc, l):
            gi, gf, gg, go = cc["g"]
            nc.vector.tensor_mul(cst[l][:], gf[:], cst[l][:])
            nc.vector.tensor_mul(gi[:], gi[:], gg[:])
            nc.vector.tensor_add(cst[l][:], cst[l][:], gi[:])
            th = gp.tile([128, U], f32, tag="th")
            nc.scalar.activation(out=th[:], in_=cst[l][:], func=AF.Tanh)
            hb = gp.tile([128, U], bf16, tag="hb")
            # halves so the first transposes / hT copies can start early
            for half in range(2):
                sl = slice(half * 256, (half + 1) * 256)
                nc.vector.tensor_mul(hb[:, sl], go[:, sl], th[:, sl])
            cc["hb"] = hb

        def transposes(cc, l):
            tpt = tp_.tile([128, 512], bf16, tag="tp")
            hb = cc["hb"]
            for half in range(2):
                for k in (2 * half, 2 * half + 1):
                    nc.tensor.transpose(tpt[:, k * 128:(k + 1) * 128],
                                        hb[:, k * 128:(k + 1) * 128], ident[:])
                sl = slice(half * 256, (half + 1) * 256)
                nc.vector.tensor_copy(out=hT[l][:, sl], in_=tpt[:, sl])

        def dense_mms(s):
            zd = dp.tile([128, D], f32, tag="zd")
            for k in range(KU):
                last = (k == KU - 1) and not with_bias
                nc.tensor.matmul(zd[:], lhsT=hT[2][:, k * 128:(k + 1) * 128],
                                 rhs=wd_sb[:, k * D:(k + 1) * D],
                                 start=(k == 0), stop=last,
                                 skip_group_check=True)
            if with_bias:
                nc.tensor.matmul(zd[:], lhsT=ones[:], rhs=bd_sb[:],
                                 start=False, stop=True, skip_group_check=True)
            # pb feeds the AR-critical pred transpose chain: emit it first
            # and on ScalarE so it runs parallel to the output copy on DVE
            pb = gp.tile([128, D], bf16, tag="pb")
            nc.scalar.activation(out=pb[:], in_=zd[:], func=AF.Copy)
            po = op.tile([128, D], f32, tag="po")
            nc.vector.tensor_copy(out=po[:], in_=zd[:])
            nc.sync.dma_start(out=out[:, s, :], in_=po[:])
            return pb

        def pred_transposes(pb, to_pred0T):
            tpt = tp_.tile([128, 512], bf16, tag="tp")
            for k in range(KD):
                nc.tensor.transpose(tpt[:, k * 128:(k + 1) * 128],
                                    pb[:, k * 128:(k + 1) * 128], ident[:])
            dst = pred0T if to_pred0T else xp.tile([128, D], bf16, tag="xc")
            nc.vector.tensor_copy(out=dst[:], in_=tpt[:, :D])
            return dst

        # ---- prologue: preload cell0's U matmuls for t=0 ----
        xc = xp.tile([128, D], bf16, tag="xc")
        for k in range(KD):
            nc.sync.dma_start(out=xc[:, k * 128:(k + 1) * 128], in_=xT[0, k])
        cc0 = cell_begin(0)
        cell_U(cc0, 0, [0, 1])
        cell_U(cc0, 0, [2, 3])

        # ---- time loop (warmup then AR), pipelined emission ----
        # period body: X(c0) T(prev c2 handled at end) ... see docstring
        for t in range(Twarm + Sout - 1):
            warm = t < Twarm
            last = t == Twarm + Sout - 2
            cell_X(cc0, 0, xc, KD, wx_sb[0])
            cell_tail(cc0, 0)
            cc1 = cell_begin(1)
            cell_U(cc1, 1, [0, 1])
            transposes(cc0, 0)                       # T0(t)
            cell_U(cc1, 1, [2, 3])
            cell_X(cc1, 1, hT[0], KU, wx_sb[1])
            cell_tail(cc1, 1)
            cc2 = cell_begin(2)
            cell_U(cc2, 2, [0, 1])
            transposes(cc1, 1)                       # T1(t)
            cell_U(cc2, 2, [2, 3])
            cell_X(cc2, 2, hT[1], KU, wx_sb[2])
            cell_tail(cc2, 2)

            do_dense = (t == 0) or not warm
            if not last:
                cc0 = cell_begin(0)
                cell_U(cc0, 0, [0, 1])
            transposes(cc2, 2)                       # T2(t)
            if do_dense:
                s_out = 0 if t == 0 else t - Twarm + 1
                pb = dense_mms(s_out)
            if not last:
                cell_U(cc0, 0, [2, 3])
            if do_dense:
                dst = pred_transposes(pb, to_pred0T=(t == 0))
                if not warm and not last:
                    xc = dst                          # AR feedback
            if warm and not last:
                # next x: from input (t+1 < Twarm) or pred0T (t+1 == Twarm)
                if t + 1 < Twarm:
                    xc = xp.tile([128, D], bf16, tag="xc")
                    for k in range(KD):
                        nc.sync.dma_start(out=xc[:, k * 128:(k + 1) * 128],
                                          in_=xT[t + 1, k])
                else:
                    xc = xp.tile([128, D], bf16, tag="xc")
                    nc.vector.tensor_copy(out=xc[:], in_=pred0T[:])

    nc.compile()
    return nc


def _get_nc(Twarm, Sout, with_bias):
    key = (Twarm, Sout, with_bias)
    if key not in _CACHE:
        _CACHE[key] = _build(Twarm, Sout, with_bias)
    return _CACHE[key]


def _prep_inputs(x, W0, U0, b0, W1, U1, b1, W2, U2, b2, Wd, bd, Twarm,
                 with_bias):
    xTa = np.ascontiguousarray(
        x.reshape(NCORES, BC, Twarm, KD, 128).transpose(0, 2, 3, 4, 1)
    ).astype(BF16)
    w0b = W0.reshape(KD, 128, G).astype(BF16)
    w1b = W1.reshape(KU, 128, G).astype(BF16)
    w2b = W2.reshape(KU, 128, G).astype(BF16)
    u0b = U0.reshape(KU, 128, G).astype(BF16)
    u1b = U1.reshape(KU, 128, G).astype(BF16)
    u2b = U2.reshape(KU, 128, G).astype(BF16)
    wdb = Wd.reshape(KU, 128, D).astype(BF16)
    idb = np.eye(128, dtype=BF16)
    maps = []
    for c in range(NCORES):
        m = {"xT": xTa[c], "w0": w0b, "w1": w1b, "w2": w2b,
             "u0": u0b, "u1": u1b, "u2": u2b, "wd": wdb, "ident": idb}
        if with_bias:
            m["b0"] = b0.reshape(1, G).astype(BF16)
            m["b1"] = b1.reshape(1, G).astype(BF16)
            m["b2"] = b2.reshape(1, G).astype(BF16)
            m["bd"] = bd.reshape(1, D).astype(BF16)
        maps.append(m)
    return maps


def kernel(x, W0, U0, b0, W1, U1, b1, W2, U2, b2, Wd, bd, **_unused):
    from concourse.bass_utils import run_bass_kernel_spmd

    x = np.asarray(x, dtype=np.float32)
    args = [np.asarray(a, dtype=np.float32)
            for a in (W0, U0, b0, W1, U1, b1, W2, U2, b2, Wd, bd)]
    W0, U0, b0, W1, U1, b1, W2, U2, b2, Wd, bd = args
    with_bias = bool(any(np.any(v) for v in (b0, b1, b2, bd)))

    nc = _get_nc(T, S, with_bias)
    in_maps = _prep_inputs(x, W0, U0, b0, W1, U1, b1, W2, U2, b2, Wd, bd,
                           T, with_bias)
    res = run_bass_kernel_spmd(nc, in_maps, core_ids=list(range(NCORES)))
    return np.concatenate([res.results[c]["out"] for c in range(NCORES)],
                          axis=0)
